# revision 22
# baseline (speedup 1.0000x reference)
"""Trainium2 Bass kernel for the Powderworld BehaviorFluidFlow step.

Contract: kernel(**inputs) takes the FULL unsharded inputs
  world         (16, 20, 512, 512) f32
  rand_movement (16, 1, 512, 512) f32
  rand_interact (16, 1, 512, 512) f32   (unused by the reference)
  rand_element  (16, 1, 512, 512) f32   (unused by the reference)
and returns the FULL (16, 20, 512, 512) f32 output.

Sharding: data-parallel over batch; core k processes batches [2k, 2k+1].
All roll-based neighbor access is along W (axis 3), which stays local.

Layout: the host packs each pixel into 12 int32 "channels":
  0: density (ch1, raw f32 bits)        1: momentum (ch6, raw f32 bits)
  2: (id, grav)     bf16 pair           3: (didg, w3)  bf16 pair
  4..10: payload bf16 pairs (w4,w5)(w7,w9)(w10,w11)(w12,w13)(w14,w15)
         (w16,w17)(w18,w19)
  11: (E, AIR) bf16 pair - host-precomputed is_element / is_air_move flags
id/grav/didg/E/AIR are small integers, exact in bf16; payload channels only
ever move (never arithmetic), so bf16 costs <= 2^-9 relative error, well
inside the 2e-2 gate.  Density and momentum feed exact f32 comparisons so
they stay f32.  E and AIR ride through the blends like payload, so the
pass-2 masks and the final is_fluid fixup never recompute set membership.

Each pass computes a single-channel move mask (a = "pixel takes its
in-direction neighbor's value", b = shifted a; disjoint), then blends all
12 i32 channels with one plain copy (DVE TensorCopy for the ten packed
channels - the Activation engine's float datapath would corrupt packed
bits - plus an ACT copy of the two f32 channels) and two copy_predicated
ops under an int8 mask broadcast across channels.  copy_predicated runs at
one ELEMENT per lane-cycle regardless of dtype, so 2 bf16 channels per i32
element double its throughput vs the f32 layout.  Mask compare chains run
on DVE; the 0/1 multiply (AND) chains and 1-column halo copies run on the
otherwise idle Pool (gpsimd) engine.

The result is stored as 11 i32 channels (E/AIR dropped) in a
tile-contiguous layout; the host unpacks back to (16, 20, 512, 512) f32.
"""
import sys

if '/opt/trn_rl_repo' not in sys.path:
    sys.path.insert(0, '/opt/trn_rl_repo')

import numpy as np
import ml_dtypes
import concourse.bacc as bacc
import concourse.mybir as mybir
import concourse.tile as tile
from concourse.bass_utils import run_bass_kernel_spmd

A = mybir.AluOpType
F32 = mybir.dt.float32
BF16 = mybir.dt.bfloat16
I32 = mybir.dt.int32
I16 = mybir.dt.int16
I8 = mybir.dt.int8

B, C, H, W = 16, 20, 512, 512
N_CORES = 8
BPC = B // N_CORES
P = 128
N_HT = H // P
NCH = 12          # i32 channels on device
NST = 11          # stored i32 channels (E/AIR dropped)
WH = W + 2        # haloed width: pixels in cols [1, W], wrap halos at 0, W+1
MAIN = slice(1, W + 1)

# i32 channel -> (lo world ch, hi world ch) for the bf16 pairs
PAIRS = [(0, 2), (8, 3), (4, 5), (7, 9), (10, 11), (12, 13), (14, 15),
         (16, 17), (18, 19)]
FLUID_IDS = (0.0, 3.0, 8.0, 9.0, 12.0, 14.0, 15.0)

_bf16 = ml_dtypes.bfloat16
_u16, _u32 = np.uint16, np.uint32

_nc_cache = {}


def build_kernel(order=2):
    key = order
    if key in _nc_cache:
        return _nc_cache[key]

    nc = bacc.Bacc("TRN2", target_bir_lowering=False, debug=False,
                   num_devices=N_CORES)
    win = nc.dram_tensor("win", [BPC, N_HT, P, NCH, W], I32,
                         kind="ExternalInput")
    rand = nc.dram_tensor("rand", [BPC, H, W], F32, kind="ExternalInput")
    out = nc.dram_tensor("out", [BPC, N_HT, P, NST, W], I32,
                         kind="ExternalOutput")

    iters = [(b, t) for b in range(BPC) for t in range(N_HT)]
    n = len(iters)
    st = [dict() for _ in range(n)]

    def bv(T, c, hi, px):
        """bf16 view of i32 channel c (hi=0 lo half / 1 hi half), pixel
        slice px."""
        return T[:].bitcast(BF16)[:, c,
                                  2 * px.start + hi:2 * (px.stop - 1) + hi + 1:2]

    with tile.TileContext(nc) as tc:
        with tc.tile_pool(name="gin", bufs=3) as ginp, \
             tc.tile_pool(name="go1", bufs=2) as go1p, \
             tc.tile_pool(name="go2", bufs=2) as go2p, \
             tc.tile_pool(name="mk", bufs=4) as mkp, \
             tc.tile_pool(name="dbl", bufs=2) as dblp, \
             tc.tile_pool(name="am8", bufs=5) as am8p, \
             tc.tile_pool(name="rp", bufs=3) as rp:

            def loads(i):
                b, t = iters[i]
                s = st[i]
                s['IN'] = ginp.tile([P, NCH, WH], I32, tag="gin", name=f"IN{i}")
                s['RAND'] = rp.tile([P, W], F32, tag="rand", name=f"RAND{i}")
                T = s['IN']
                nc.sync.dma_start(T[:, :, MAIN], win[b, t, :, :, :])
                nc.sync.dma_start(s['RAND'][:], rand[b, t * P:(t + 1) * P, :])
                nc.vector.tensor_copy(T[:, :, 0:1], T[:, :, W:W + 1])
                nc.vector.tensor_copy(T[:, :, W + 1:W + 2], T[:, :, 1:2])

            def mask_pass(i, which):
                """Move mask for a pass -> AMf (f32, haloed) + AM8 (int8).

                which=1: in-dir nbr = x-1 (cols 0:W), overlap shift = x+1.
                which=2: in-dir nbr = x+1 (cols 2:W+2), overlap shift = x-1.
                """
                s = st[i]
                cur = s['IN'] if which == 1 else s['O1']
                nbr = slice(0, W) if which == 1 else slice(2, W + 2)
                curf = cur[:].bitcast(F32)
                FS = mkp.tile([P, W], F32, tag="mk")
                DN = mkp.tile([P, W], F32, tag="mk")
                NDG = mkp.tile([P, W], F32, tag="mk")
                GB = mkp.tile([P, W], F32, tag="mk")
                DBL = dblp.tile([P, WH], F32, tag="dbl")
                AM8 = am8p.tile([P, WH], I8, tag="am8", name=f"AM8{which}_{i}")

                # fall score: rand + momentum (+ 2*b1 after pass 1)
                nc.gpsimd.tensor_tensor(FS[:], s['RAND'][:], curf[:, 1, MAIN],
                                        A.add)
                if which == 2:
                    nc.vector.scalar_tensor_tensor(
                        FS[:], s['A18'][:, 2:W + 2], 2.0, FS[:], A.mult, A.add)
                # density strictly lower in move direction
                nc.vector.tensor_tensor(DN[:], curf[:, 0, MAIN],
                                        curf[:, 0, nbr], A.is_gt)
                cmp_op = A.is_gt if which == 1 else A.is_le
                nc.vector.scalar_tensor_tensor(FS[:], FS[:], 0.5, DN[:],
                                               cmp_op, A.logical_and)
                # not-did-gravity | air-move
                nc.vector.scalar_tensor_tensor(NDG[:], bv(cur, 3, 0, MAIN),
                                               0.5, bv(cur, 11, 1, MAIN),
                                               A.is_lt, A.logical_or)
                # chain the 0/1 ANDs (multiplies) on Pool
                nc.gpsimd.tensor_tensor(NDG[:], bv(cur, 11, 0, MAIN), NDG[:],
                                        A.mult)
                nc.gpsimd.tensor_tensor(FS[:], FS[:], NDG[:], A.mult)
                nc.gpsimd.tensor_tensor(GB[:], bv(cur, 2, 1, MAIN),
                                        bv(cur, 2, 1, nbr), A.mult)
                nc.gpsimd.tensor_tensor(DBL[:, MAIN], FS[:], GB[:], A.mult)
                # overlap kill: a = dbl & ~shift(dbl), written straight to int8
                if which == 1:
                    nc.scalar.copy(DBL[:, W + 1:W + 2], DBL[:, 1:2])
                    nc.vector.scalar_tensor_tensor(
                        AM8[:, MAIN], DBL[:, 2:W + 2], 0.0, DBL[:, MAIN],
                        A.is_equal, A.logical_and)
                else:
                    nc.scalar.copy(DBL[:, 0:1], DBL[:, W:W + 1])
                    nc.vector.scalar_tensor_tensor(
                        AM8[:, MAIN], DBL[:, 0:W], 0.0, DBL[:, MAIN],
                        A.is_equal, A.logical_and)
                nc.scalar.copy(AM8[:, 0:1], AM8[:, W:W + 1])
                nc.scalar.copy(AM8[:, W + 1:W + 2], AM8[:, 1:2])
                s[f'A{which}8'] = AM8

            def blend(i, which):
                s = st[i]
                if which == 1:
                    src, dpool, dname = s['IN'], go1p, "go1"
                else:
                    src, dpool, dname = s['O1'], go2p, "go2"
                dst = dpool.tile([P, NCH, WH], I32, tag=dname,
                                 name=f"O{which}_{i}")
                s[f'O{which}'] = dst
                AM8 = s[f'A{which}8']
                if which == 1:
                    a_src, b_msk, b_src = slice(0, W), slice(2, W + 2), slice(2, W + 2)
                else:
                    a_src, b_msk, b_src = slice(2, W + 2), slice(0, W), slice(0, W)
                # plain copy on ACT as int16 (i16 -> f32 -> i16 is lossless, so
                # packed bf16 pairs and raw f32 bits survive), split in halves
                # so each predicated pair can start as soon as its half lands
                for c0, c1 in ((0, NCH // 2), (NCH // 2, NCH)):
                    nc.scalar.copy(dst[:, c0:c1, MAIN].bitcast(I16),
                                   src[:, c0:c1, MAIN].bitcast(I16))
                    nch = c1 - c0
                    am = AM8[:, MAIN].unsqueeze(1).broadcast_to((P, nch, W))
                    bm = AM8[:, b_msk].unsqueeze(1).broadcast_to((P, nch, W))
                    nc.vector.copy_predicated(dst[:, c0:c1, MAIN], am,
                                              src[:, c0:c1, a_src])
                    nc.vector.copy_predicated(dst[:, c0:c1, MAIN], bm,
                                              src[:, c0:c1, b_src])
                if which == 1:
                    nc.vector.tensor_copy(dst[:, :, 0:1], dst[:, :, W:W + 1])
                    nc.vector.tensor_copy(dst[:, :, W + 1:W + 2], dst[:, :, 1:2])
                else:
                    b, t = iters[i]
                    nc.sync.dma_start(out[b, t, :, 2:NST, :],
                                      dst[:, 2:NST, MAIN])
                    nc.sync.dma_start(out[b, t, :, 0:1, :], dst[:, 0:1, MAIN])

            def fixup(i):
                b, t = iters[i]
                s = st[i]
                O2 = s['O2']
                NF = mkp.tile([P, W], F32, tag="mk")
                FLI = am8p.tile([P, W], I8, tag="am8", name=f"FLI{i}")
                # nfm = 2*b1 - 2*b2 (masks exactly 0/1)
                nc.gpsimd.tensor_tensor(NF[:], s['A18'][:, 2:W + 2],
                                        s['A28'][:, 0:W], A.subtract)
                nc.gpsimd.tensor_scalar(NF[:], NF[:], 2.0, None, A.mult)
                nc.vector.tensor_copy(FLI[:], bv(O2, 11, 0, MAIN))
                nc.vector.copy_predicated(O2[:].bitcast(F32)[:, 1, MAIN],
                                          FLI[:], NF[:])
                nc.sync.dma_start(out[b, t, :, 1:2, :], O2[:, 1:2, MAIN])

            # ---- software-pipelined emission -------------------------------
            if order == 1:
                loads(0)
                loads(1)
                for i in range(n):
                    if i + 2 < n:
                        loads(i + 2)
                    mask_pass(i, 1)
                    blend(i, 1)
                    mask_pass(i, 2)
                    blend(i, 2)
                    fixup(i)
            elif order == 2:
                # two iterations' chains interleaved: mask chain of i+1 runs
                # while the blends of i occupy DVE/ACT
                loads(0)
                loads(1)
                mask_pass(0, 1)
                for i in range(n):
                    if i + 2 < n:
                        loads(i + 2)
                    blend(i, 1)
                    if i + 1 < n:
                        mask_pass(i + 1, 1)
                    mask_pass(i, 2)
                    blend(i, 2)
                    fixup(i)
            elif order == 4:
                # order 2, with the previous iteration's fixup deferred so it
                # fills DVE gaps during the next mask chain
                loads(0)
                loads(1)
                mask_pass(0, 1)
                for i in range(n):
                    if i + 2 < n:
                        loads(i + 2)
                    blend(i, 1)
                    if i + 1 < n:
                        mask_pass(i + 1, 1)
                    mask_pass(i, 2)
                    blend(i, 2)
                    if i > 0:
                        fixup(i - 1)
                fixup(n - 1)
            else:
                loads(0)
                loads(1)
                mask_pass(0, 1)
                blend(0, 1)
                for i in range(n):
                    if i + 2 < n:
                        loads(i + 2)
                    mask_pass(i, 2)
                    if i + 1 < n:
                        mask_pass(i + 1, 1)
                    blend(i, 2)
                    if i + 1 < n:
                        blend(i + 1, 1)
                    fixup(i)

    nc.compile()
    _nc_cache[key] = nc
    return nc


def _pack(lo, hi):
    lo16 = np.ascontiguousarray(lo).astype(_bf16).view(_u16).astype(_u32)
    hi16 = np.ascontiguousarray(hi).astype(_bf16).view(_u16).astype(_u32)
    return lo16 | (hi16 << 16)


def prepare_inputs(world, rand_movement):
    """Pack the full-batch inputs into the device layout.

    Returns {"win": (B, N_HT, P, NCH, W) int32, "rand": (B, H, W) f32}.
    """
    ids = world[:, 0]
    E = np.zeros(ids.shape, np.float32)
    for v in FLUID_IDS:
        E += (ids == v)
    AIR = ((ids == 14.0) | (ids == 15.0)).astype(np.float32)
    ch = np.empty((B, NCH, H, W), _u32)
    ch[:, 0] = np.ascontiguousarray(world[:, 1]).view(_u32)
    ch[:, 1] = np.ascontiguousarray(world[:, 6]).view(_u32)
    for j, (lo, hi) in enumerate(PAIRS):
        ch[:, 2 + j] = _pack(world[:, lo], world[:, hi])
    ch[:, 11] = _pack(E, AIR)
    win = np.ascontiguousarray(
        ch.reshape(B, NCH, N_HT, P, W).transpose(0, 2, 3, 1, 4)).view(np.int32)
    return {"win": win, "rand": np.ascontiguousarray(rand_movement[:, 0])}


def unpack_output(stored):
    """(B, N_HT, P, NST, W) int32 device output -> (B, C, H, W) f32."""
    oc = stored.view(_u32).transpose(0, 3, 1, 2, 4).reshape(B, NST, H, W)
    full = np.empty((B, C, H, W), np.float32)
    full[:, 1] = np.ascontiguousarray(oc[:, 0]).view(np.float32)
    full[:, 6] = np.ascontiguousarray(oc[:, 1]).view(np.float32)
    for j, (lo, hi) in enumerate(PAIRS):
        c = np.ascontiguousarray(oc[:, 2 + j])
        full[:, lo] = (c & 0xFFFF).astype(_u16).view(_bf16).astype(np.float32)
        full[:, hi] = (c >> 16).astype(_u16).view(_bf16).astype(np.float32)
    return full


def kernel(world, rand_movement, rand_interact, rand_element):
    del rand_interact, rand_element
    nc = build_kernel()
    packed = prepare_inputs(np.asarray(world), np.asarray(rand_movement))
    in_maps = []
    for k in range(N_CORES):
        bs = slice(k * BPC, (k + 1) * BPC)
        in_maps.append({"win": packed["win"][bs], "rand": packed["rand"][bs]})
    res = run_bass_kernel_spmd(nc, in_maps, list(range(N_CORES)))
    stored = np.concatenate([res.results[k]["out"] for k in range(N_CORES)],
                            axis=0)
    return unpack_output(stored)


# revision 27
# speedup vs baseline: 1.1840x; 1.1840x over previous
"""Trainium2 Bass kernel for the Powderworld BehaviorFluidFlow step.

Contract: kernel(**inputs) takes the FULL unsharded inputs
  world         (16, 20, 512, 512) f32
  rand_movement (16, 1, 512, 512) f32
  rand_interact (16, 1, 512, 512) f32   (unused by the reference)
  rand_element  (16, 1, 512, 512) f32   (unused by the reference)
and returns the FULL (16, 20, 512, 512) f32 output.

Sharding: data-parallel over batch; core k processes batches [2k, 2k+1].
All roll-based neighbor access is along W (axis 3), which stays local.

Layout: the host packs each pixel into 12 int32 "channels":
  0: density (ch1, raw f32 bits)        1: momentum (ch6, raw f32 bits)
  2: (id, grav)     bf16 pair           3: (didg, w3)  bf16 pair
  4..10: payload bf16 pairs (w4,w5)(w7,w9)(w10,w11)(w12,w13)(w14,w15)
         (w16,w17)(w18,w19)
  11: (E, AIR) bf16 pair - host-precomputed is_element / is_air_move flags
id/grav/didg/E/AIR are small integers, exact in bf16; payload channels only
ever move (never arithmetic), so bf16 costs <= 2^-9 relative error, well
inside the 2e-2 gate.  Density and momentum feed exact f32 comparisons so
they stay f32.  E and AIR ride through the blends like payload, so the
pass-2 masks and the final is_fluid fixup never recompute set membership.

Each pass computes a single-channel move mask (a = "pixel takes its
in-direction neighbor's value", b = shifted a; disjoint, written straight
to int8 by the overlap-kill op), then blends all 12 i32 channels: a plain
copy on the Activation engine through int16 bitcast views (i16 -> f32 ->
i16 is lossless, unlike i32, through ACT's float datapath) plus two DVE
copy_predicated ops under the int8 mask broadcast across channels, split
into channel halves so each predicated pair starts as soon as its half is
copied.  copy_predicated runs at one ELEMENT per lane-cycle regardless of
dtype, so 2 bf16 channels per i32 element double its throughput vs the
f32 layout.  Mask compares run on DVE, the 0/1 multiply (AND) chains on
the Pool (gpsimd) engine; emission interleaves two iterations so the
pass-1 mask chain of iteration i+1 overlaps the blends of iteration i.

The result is stored as 11 i32 channels (E/AIR dropped) in a
tile-contiguous layout; the host unpacks back to (16, 20, 512, 512) f32.
"""
import sys

if '/opt/trn_rl_repo' not in sys.path:
    sys.path.insert(0, '/opt/trn_rl_repo')

import numpy as np
try:
    import ml_dtypes
    _BF = ml_dtypes.bfloat16
except ImportError:          # pragma: no cover - pure-numpy fallback
    _BF = None
import concourse.bacc as bacc
import concourse.mybir as mybir
import concourse.tile as tile
from concourse.bass_utils import run_bass_kernel_spmd

A = mybir.AluOpType
F32 = mybir.dt.float32
BF16 = mybir.dt.bfloat16
I32 = mybir.dt.int32
I16 = mybir.dt.int16
I8 = mybir.dt.int8

B, C, H, W = 16, 20, 512, 512
N_CORES = 8
BPC = B // N_CORES
P = 128
N_HT = H // P
NCH = 12          # i32 channels on device
NST = 11          # stored i32 channels (E/AIR dropped)
WH = W + 2        # haloed width: pixels in cols [1, W], wrap halos at 0, W+1
MAIN = slice(1, W + 1)

# i32 channel -> (lo world ch, hi world ch) for the bf16 pairs
PAIRS = [(0, 2), (8, 3), (4, 5), (7, 9), (10, 11), (12, 13), (14, 15),
         (16, 17), (18, 19)]
FLUID_IDS = (0.0, 3.0, 8.0, 9.0, 12.0, 14.0, 15.0)

_u16, _u32 = np.uint16, np.uint32


def _f32_to_bf16_bits(x):
    """f32 -> bf16 bit pattern (uint16), round-to-nearest-even."""
    if _BF is not None:
        return np.ascontiguousarray(x, np.float32).astype(_BF).view(_u16)
    v = np.ascontiguousarray(x, np.float32).view(_u32)
    return ((v + 0x7FFF + ((v >> 16) & 1)) >> 16).astype(_u16)


def _bf16_bits_to_f32(b):
    """bf16 bit pattern (uint16) -> f32."""
    return (b.astype(_u32) << 16).view(np.float32)

_nc_cache = {}


def build_kernel(order=2):
    key = order
    if key in _nc_cache:
        return _nc_cache[key]

    nc = bacc.Bacc("TRN2", target_bir_lowering=False, debug=False,
                   num_devices=N_CORES)
    win = nc.dram_tensor("win", [BPC, N_HT, P, NCH, W], I32,
                         kind="ExternalInput")
    rand = nc.dram_tensor("rand", [BPC, H, W], F32, kind="ExternalInput")
    out = nc.dram_tensor("out", [BPC, N_HT, P, NST, W], I32,
                         kind="ExternalOutput")

    iters = [(b, t) for b in range(BPC) for t in range(N_HT)]
    n = len(iters)
    st = [dict() for _ in range(n)]

    def bv(T, c, hi, px):
        """bf16 view of i32 channel c (hi=0 lo half / 1 hi half), pixel
        slice px."""
        return T[:].bitcast(BF16)[:, c,
                                  2 * px.start + hi:2 * (px.stop - 1) + hi + 1:2]

    with tile.TileContext(nc) as tc:
        with tc.tile_pool(name="gin", bufs=3) as ginp, \
             tc.tile_pool(name="go1", bufs=2) as go1p, \
             tc.tile_pool(name="go2", bufs=2) as go2p, \
             tc.tile_pool(name="mk", bufs=4) as mkp, \
             tc.tile_pool(name="dbl", bufs=2) as dblp, \
             tc.tile_pool(name="am8", bufs=5) as am8p, \
             tc.tile_pool(name="rp", bufs=3) as rp:

            def loads(i):
                b, t = iters[i]
                s = st[i]
                s['IN'] = ginp.tile([P, NCH, WH], I32, tag="gin", name=f"IN{i}")
                s['RAND'] = rp.tile([P, W], F32, tag="rand", name=f"RAND{i}")
                T = s['IN']
                nc.sync.dma_start(T[:, :, MAIN], win[b, t, :, :, :])
                nc.sync.dma_start(s['RAND'][:], rand[b, t * P:(t + 1) * P, :])
                nc.vector.tensor_copy(T[:, :, 0:1], T[:, :, W:W + 1])
                nc.vector.tensor_copy(T[:, :, W + 1:W + 2], T[:, :, 1:2])

            def mask_pass(i, which):
                """Move mask for a pass -> AMf (f32, haloed) + AM8 (int8).

                which=1: in-dir nbr = x-1 (cols 0:W), overlap shift = x+1.
                which=2: in-dir nbr = x+1 (cols 2:W+2), overlap shift = x-1.
                """
                s = st[i]
                cur = s['IN'] if which == 1 else s['O1']
                nbr = slice(0, W) if which == 1 else slice(2, W + 2)
                curf = cur[:].bitcast(F32)
                FS = mkp.tile([P, W], F32, tag="mk")
                DN = mkp.tile([P, W], F32, tag="mk")
                NDG = mkp.tile([P, W], F32, tag="mk")
                GB = mkp.tile([P, W], F32, tag="mk")
                DBL = dblp.tile([P, WH], F32, tag="dbl")
                AM8 = am8p.tile([P, WH], I8, tag="am8", name=f"AM8{which}_{i}")

                # fall score: rand + momentum (+ 2*b1 after pass 1)
                nc.gpsimd.tensor_tensor(FS[:], s['RAND'][:], curf[:, 1, MAIN],
                                        A.add)
                if which == 2:
                    nc.vector.scalar_tensor_tensor(
                        FS[:], s['A18'][:, 2:W + 2], 2.0, FS[:], A.mult, A.add)
                # density strictly lower in move direction
                nc.vector.tensor_tensor(DN[:], curf[:, 0, MAIN],
                                        curf[:, 0, nbr], A.is_gt)
                cmp_op = A.is_gt if which == 1 else A.is_le
                nc.vector.scalar_tensor_tensor(FS[:], FS[:], 0.5, DN[:],
                                               cmp_op, A.logical_and)
                # not-did-gravity | air-move
                nc.vector.scalar_tensor_tensor(NDG[:], bv(cur, 3, 0, MAIN),
                                               0.5, bv(cur, 11, 1, MAIN),
                                               A.is_lt, A.logical_or)
                # chain the 0/1 ANDs (multiplies) on Pool
                nc.gpsimd.tensor_tensor(NDG[:], bv(cur, 11, 0, MAIN), NDG[:],
                                        A.mult)
                nc.gpsimd.tensor_tensor(FS[:], FS[:], NDG[:], A.mult)
                nc.gpsimd.tensor_tensor(GB[:], bv(cur, 2, 1, MAIN),
                                        bv(cur, 2, 1, nbr), A.mult)
                nc.gpsimd.tensor_tensor(DBL[:, MAIN], FS[:], GB[:], A.mult)
                # overlap kill: a = dbl & ~shift(dbl), written straight to int8
                if which == 1:
                    nc.scalar.copy(DBL[:, W + 1:W + 2], DBL[:, 1:2])
                    nc.vector.scalar_tensor_tensor(
                        AM8[:, MAIN], DBL[:, 2:W + 2], 0.0, DBL[:, MAIN],
                        A.is_equal, A.logical_and)
                else:
                    nc.scalar.copy(DBL[:, 0:1], DBL[:, W:W + 1])
                    nc.vector.scalar_tensor_tensor(
                        AM8[:, MAIN], DBL[:, 0:W], 0.0, DBL[:, MAIN],
                        A.is_equal, A.logical_and)
                nc.scalar.copy(AM8[:, 0:1], AM8[:, W:W + 1])
                nc.scalar.copy(AM8[:, W + 1:W + 2], AM8[:, 1:2])
                s[f'A{which}8'] = AM8

            def blend(i, which):
                s = st[i]
                if which == 1:
                    src, dpool, dname = s['IN'], go1p, "go1"
                else:
                    src, dpool, dname = s['O1'], go2p, "go2"
                dst = dpool.tile([P, NCH, WH], I32, tag=dname,
                                 name=f"O{which}_{i}")
                s[f'O{which}'] = dst
                AM8 = s[f'A{which}8']
                if which == 1:
                    a_src, b_msk, b_src = slice(0, W), slice(2, W + 2), slice(2, W + 2)
                else:
                    a_src, b_msk, b_src = slice(2, W + 2), slice(0, W), slice(0, W)
                # plain copy on ACT as int16 (i16 -> f32 -> i16 is lossless, so
                # packed bf16 pairs and raw f32 bits survive), split in halves
                # so each predicated pair can start as soon as its half lands
                for c0, c1 in ((0, NCH // 2), (NCH // 2, NCH)):
                    nc.scalar.copy(dst[:, c0:c1, MAIN].bitcast(I16),
                                   src[:, c0:c1, MAIN].bitcast(I16))
                    nch = c1 - c0
                    am = AM8[:, MAIN].unsqueeze(1).broadcast_to((P, nch, W))
                    bm = AM8[:, b_msk].unsqueeze(1).broadcast_to((P, nch, W))
                    nc.vector.copy_predicated(dst[:, c0:c1, MAIN], am,
                                              src[:, c0:c1, a_src])
                    nc.vector.copy_predicated(dst[:, c0:c1, MAIN], bm,
                                              src[:, c0:c1, b_src])
                if which == 1:
                    nc.vector.tensor_copy(dst[:, :, 0:1], dst[:, :, W:W + 1])
                    nc.vector.tensor_copy(dst[:, :, W + 1:W + 2], dst[:, :, 1:2])
                else:
                    b, t = iters[i]
                    nc.sync.dma_start(out[b, t, :, 2:NST, :],
                                      dst[:, 2:NST, MAIN])
                    nc.sync.dma_start(out[b, t, :, 0:1, :], dst[:, 0:1, MAIN])

            def fixup(i):
                b, t = iters[i]
                s = st[i]
                O2 = s['O2']
                NF = mkp.tile([P, W], F32, tag="mk")
                FLI = am8p.tile([P, W], I8, tag="am8", name=f"FLI{i}")
                # nfm = 2*b1 - 2*b2 (masks exactly 0/1)
                nc.gpsimd.tensor_tensor(NF[:], s['A18'][:, 2:W + 2],
                                        s['A28'][:, 0:W], A.subtract)
                nc.gpsimd.tensor_scalar(NF[:], NF[:], 2.0, None, A.mult)
                nc.vector.tensor_copy(FLI[:], bv(O2, 11, 0, MAIN))
                nc.vector.copy_predicated(O2[:].bitcast(F32)[:, 1, MAIN],
                                          FLI[:], NF[:])
                nc.sync.dma_start(out[b, t, :, 1:2, :], O2[:, 1:2, MAIN])

            # ---- software-pipelined emission -------------------------------
            if order == 1:
                loads(0)
                loads(1)
                for i in range(n):
                    if i + 2 < n:
                        loads(i + 2)
                    mask_pass(i, 1)
                    blend(i, 1)
                    mask_pass(i, 2)
                    blend(i, 2)
                    fixup(i)
            elif order == 2:
                # two iterations' chains interleaved: mask chain of i+1 runs
                # while the blends of i occupy DVE/ACT
                loads(0)
                loads(1)
                mask_pass(0, 1)
                for i in range(n):
                    if i + 2 < n:
                        loads(i + 2)
                    blend(i, 1)
                    if i + 1 < n:
                        mask_pass(i + 1, 1)
                    mask_pass(i, 2)
                    blend(i, 2)
                    fixup(i)
            elif order == 4:
                # order 2, with the previous iteration's fixup deferred so it
                # fills DVE gaps during the next mask chain
                loads(0)
                loads(1)
                mask_pass(0, 1)
                for i in range(n):
                    if i + 2 < n:
                        loads(i + 2)
                    blend(i, 1)
                    if i + 1 < n:
                        mask_pass(i + 1, 1)
                    mask_pass(i, 2)
                    blend(i, 2)
                    if i > 0:
                        fixup(i - 1)
                fixup(n - 1)
            else:
                loads(0)
                loads(1)
                mask_pass(0, 1)
                blend(0, 1)
                for i in range(n):
                    if i + 2 < n:
                        loads(i + 2)
                    mask_pass(i, 2)
                    if i + 1 < n:
                        mask_pass(i + 1, 1)
                    blend(i, 2)
                    if i + 1 < n:
                        blend(i + 1, 1)
                    fixup(i)

    nc.compile()
    _nc_cache[key] = nc
    return nc


def _pack(lo, hi):
    return (_f32_to_bf16_bits(lo).astype(_u32)
            | (_f32_to_bf16_bits(hi).astype(_u32) << 16))


def prepare_inputs(world, rand_movement):
    """Pack the full-batch inputs into the device layout.

    Returns {"win": (B, N_HT, P, NCH, W) int32, "rand": (B, H, W) f32}.
    """
    ids = world[:, 0]
    E = np.zeros(ids.shape, np.float32)
    for v in FLUID_IDS:
        E += (ids == v)
    AIR = ((ids == 14.0) | (ids == 15.0)).astype(np.float32)
    ch = np.empty((B, NCH, H, W), _u32)
    ch[:, 0] = np.ascontiguousarray(world[:, 1]).view(_u32)
    ch[:, 1] = np.ascontiguousarray(world[:, 6]).view(_u32)
    for j, (lo, hi) in enumerate(PAIRS):
        ch[:, 2 + j] = _pack(world[:, lo], world[:, hi])
    ch[:, 11] = _pack(E, AIR)
    win = np.ascontiguousarray(
        ch.reshape(B, NCH, N_HT, P, W).transpose(0, 2, 3, 1, 4)).view(np.int32)
    return {"win": win, "rand": np.ascontiguousarray(rand_movement[:, 0])}


def unpack_output(stored):
    """(B, N_HT, P, NST, W) int32 device output -> (B, C, H, W) f32."""
    oc = stored.view(_u32).transpose(0, 3, 1, 2, 4).reshape(B, NST, H, W)
    full = np.empty((B, C, H, W), np.float32)
    full[:, 1] = np.ascontiguousarray(oc[:, 0]).view(np.float32)
    full[:, 6] = np.ascontiguousarray(oc[:, 1]).view(np.float32)
    for j, (lo, hi) in enumerate(PAIRS):
        c = np.ascontiguousarray(oc[:, 2 + j])
        full[:, lo] = _bf16_bits_to_f32((c & 0xFFFF).astype(_u16))
        full[:, hi] = _bf16_bits_to_f32((c >> 16).astype(_u16))
    return full


def kernel(world, rand_movement, rand_interact, rand_element):
    del rand_interact, rand_element
    nc = build_kernel()
    packed = prepare_inputs(np.asarray(world), np.asarray(rand_movement))
    in_maps = []
    for k in range(N_CORES):
        bs = slice(k * BPC, (k + 1) * BPC)
        in_maps.append({"win": packed["win"][bs], "rand": packed["rand"][bs]})
    res = run_bass_kernel_spmd(nc, in_maps, list(range(N_CORES)))
    stored = np.concatenate([res.results[k]["out"] for k in range(N_CORES)],
                            axis=0)
    return unpack_output(stored)


# revision 30
# speedup vs baseline: 2.1461x; 1.8125x over previous
"""Trainium2 Bass kernel for the Powderworld BehaviorFluidFlow step.

Contract: kernel(**inputs) takes the FULL unsharded inputs
  world         (16, 20, 512, 512) f32
  rand_movement (16, 1, 512, 512) f32
  rand_interact (16, 1, 512, 512) f32   (unused by the reference)
  rand_element  (16, 1, 512, 512) f32   (unused by the reference)
and returns the FULL (16, 20, 512, 512) f32 output.

Sharding: data-parallel over batch; core k processes batches [2k, 2k+1].
All roll-based neighbor access is along W (axis 3), which stays local.

Layout: the host packs each pixel into 12 int32 "channels":
  0: density (ch1, raw f32 bits)        1: momentum (ch6, raw f32 bits)
  2: (id, grav)     bf16 pair           3: (didg, w3)  bf16 pair
  4..10: payload bf16 pairs (w4,w5)(w7,w9)(w10,w11)(w12,w13)(w14,w15)
         (w16,w17)(w18,w19)
  11: (E, AIR) bf16 pair - host-precomputed is_element / is_air_move flags
id/grav/didg/E/AIR are small integers, exact in bf16; payload channels only
ever move (never arithmetic), so bf16 costs <= 2^-9 relative error, well
inside the 2e-2 gate.  Density and momentum feed exact f32 comparisons so
they stay f32.  E and AIR ride through the blends like payload, so the
pass-2 masks and the final is_fluid fixup never recompute set membership.

Each pass computes a single-channel move mask (a = "pixel takes its
in-direction neighbor's value", b = shifted a; disjoint, written straight
to int8 by the overlap-kill op), then blends all 12 i32 channels: a plain
copy on the Activation engine through int16 bitcast views (i16 -> f32 ->
i16 is lossless, unlike i32, through ACT's float datapath) plus two DVE
copy_predicated ops under the int8 mask broadcast across channels, split
into channel halves so each predicated pair starts as soon as its half is
copied.  copy_predicated runs at one ELEMENT per lane-cycle regardless of
dtype, so 2 bf16 channels per i32 element double its throughput vs the
f32 layout.  Mask compares run on DVE, the 0/1 multiply (AND) chains on
the Pool (gpsimd) engine; emission interleaves two iterations so the
pass-1 mask chain of iteration i+1 overlaps the blends of iteration i.

The result is stored as 11 i32 channels (E/AIR dropped) in a
tile-contiguous layout; the host unpacks back to (16, 20, 512, 512) f32.
"""
import sys

if '/opt/trn_rl_repo' not in sys.path:
    sys.path.insert(0, '/opt/trn_rl_repo')

import numpy as np
try:
    import ml_dtypes
    _BF = ml_dtypes.bfloat16
except ImportError:          # pragma: no cover - pure-numpy fallback
    _BF = None
import concourse.bacc as bacc
import concourse.mybir as mybir
import concourse.tile as tile
from concourse.bass_utils import run_bass_kernel_spmd

A = mybir.AluOpType
F32 = mybir.dt.float32
BF16 = mybir.dt.bfloat16
I32 = mybir.dt.int32
I16 = mybir.dt.int16
I8 = mybir.dt.int8

B, C, H, W = 16, 20, 512, 512
N_CORES = 8
BPC = B // N_CORES
P = 128
N_HT = H // P
NCH = 12          # i32 channels on device
NST = 11          # stored i32 channels (E/AIR dropped)
WH = W + 2        # haloed width: pixels in cols [1, W], wrap halos at 0, W+1
MAIN = slice(1, W + 1)

# i32 channel -> (lo world ch, hi world ch) for the bf16 pairs
PAIRS = [(0, 2), (8, 3), (4, 5), (7, 9), (10, 11), (12, 13), (14, 15),
         (16, 17), (18, 19)]
FLUID_IDS = (0.0, 3.0, 8.0, 9.0, 12.0, 14.0, 15.0)

_u16, _u32 = np.uint16, np.uint32


def _f32_to_bf16_bits(x):
    """f32 -> bf16 bit pattern (uint16), round-to-nearest-even."""
    if _BF is not None:
        return np.ascontiguousarray(x, np.float32).astype(_BF).view(_u16)
    v = np.ascontiguousarray(x, np.float32).view(_u32)
    return ((v + 0x7FFF + ((v >> 16) & 1)) >> 16).astype(_u16)


def _bf16_bits_to_f32(b):
    """bf16 bit pattern (uint16) -> f32."""
    return (b.astype(_u32) << 16).view(np.float32)

_nc_cache = {}


def build_kernel(order=2, bufs=(2, 3, 2, 6, 3, 6, 3)):
    key = (order, bufs)
    if key in _nc_cache:
        return _nc_cache[key]
    b_gin, b_go1, b_go2, b_mk, b_dbl, b_am8, b_rp = bufs

    nc = bacc.Bacc("TRN2", target_bir_lowering=False, debug=False,
                   num_devices=N_CORES)
    win = nc.dram_tensor("win", [BPC, N_HT, P, NCH, W], I32,
                         kind="ExternalInput")
    rand = nc.dram_tensor("rand", [BPC, H, W], F32, kind="ExternalInput")
    out = nc.dram_tensor("out", [BPC, N_HT, P, NST, W], I32,
                         kind="ExternalOutput")

    iters = [(b, t) for b in range(BPC) for t in range(N_HT)]
    n = len(iters)
    st = [dict() for _ in range(n)]

    def bv(T, c, hi, px):
        """bf16 view of i32 channel c (hi=0 lo half / 1 hi half), pixel
        slice px."""
        return T[:].bitcast(BF16)[:, c,
                                  2 * px.start + hi:2 * (px.stop - 1) + hi + 1:2]

    with tile.TileContext(nc) as tc:
        with tc.tile_pool(name="gin", bufs=b_gin) as ginp, \
             tc.tile_pool(name="go1", bufs=b_go1) as go1p, \
             tc.tile_pool(name="go2", bufs=b_go2) as go2p, \
             tc.tile_pool(name="mk", bufs=b_mk) as mkp, \
             tc.tile_pool(name="dbl", bufs=b_dbl) as dblp, \
             tc.tile_pool(name="am8", bufs=b_am8) as am8p, \
             tc.tile_pool(name="rp", bufs=b_rp) as rp:

            def loads(i):
                b, t = iters[i]
                s = st[i]
                s['IN'] = ginp.tile([P, NCH, WH], I32, tag="gin", name=f"IN{i}")
                s['RAND'] = rp.tile([P, W], F32, tag="rand", name=f"RAND{i}")
                T = s['IN']
                nc.sync.dma_start(T[:, :, MAIN], win[b, t, :, :, :])
                nc.sync.dma_start(s['RAND'][:], rand[b, t * P:(t + 1) * P, :])
                nc.vector.tensor_copy(T[:, :, 0:1], T[:, :, W:W + 1])
                nc.vector.tensor_copy(T[:, :, W + 1:W + 2], T[:, :, 1:2])

            def mask_pass(i, which):
                """Move mask for a pass -> AMf (f32, haloed) + AM8 (int8).

                which=1: in-dir nbr = x-1 (cols 0:W), overlap shift = x+1.
                which=2: in-dir nbr = x+1 (cols 2:W+2), overlap shift = x-1.
                """
                s = st[i]
                cur = s['IN'] if which == 1 else s['O1']
                nbr = slice(0, W) if which == 1 else slice(2, W + 2)
                curf = cur[:].bitcast(F32)
                FS = mkp.tile([P, W], F32, tag="mk")
                DN = mkp.tile([P, W], F32, tag="mk")
                NDG = mkp.tile([P, W], F32, tag="mk")
                GB = mkp.tile([P, W], F32, tag="mk")
                DBL = dblp.tile([P, WH], F32, tag="dbl")
                AM8 = am8p.tile([P, WH], I8, tag="am8", name=f"AM8{which}_{i}")

                # fall score: rand + momentum (+ 2*b1 after pass 1)
                nc.gpsimd.tensor_tensor(FS[:], s['RAND'][:], curf[:, 1, MAIN],
                                        A.add)
                if which == 2:
                    nc.vector.scalar_tensor_tensor(
                        FS[:], s['A18'][:, 2:W + 2], 2.0, FS[:], A.mult, A.add)
                # density strictly lower in move direction
                nc.vector.tensor_tensor(DN[:], curf[:, 0, MAIN],
                                        curf[:, 0, nbr], A.is_gt)
                cmp_op = A.is_gt if which == 1 else A.is_le
                nc.vector.scalar_tensor_tensor(FS[:], FS[:], 0.5, DN[:],
                                               cmp_op, A.logical_and)
                # not-did-gravity | air-move
                nc.vector.scalar_tensor_tensor(NDG[:], bv(cur, 3, 0, MAIN),
                                               0.5, bv(cur, 11, 1, MAIN),
                                               A.is_lt, A.logical_or)
                # chain the 0/1 ANDs (multiplies) on Pool
                nc.gpsimd.tensor_tensor(NDG[:], bv(cur, 11, 0, MAIN), NDG[:],
                                        A.mult)
                nc.gpsimd.tensor_tensor(FS[:], FS[:], NDG[:], A.mult)
                nc.gpsimd.tensor_tensor(GB[:], bv(cur, 2, 1, MAIN),
                                        bv(cur, 2, 1, nbr), A.mult)
                nc.gpsimd.tensor_tensor(DBL[:, MAIN], FS[:], GB[:], A.mult)
                # overlap kill: a = dbl & ~shift(dbl), written straight to int8
                if which == 1:
                    nc.scalar.copy(DBL[:, W + 1:W + 2], DBL[:, 1:2])
                    nc.vector.scalar_tensor_tensor(
                        AM8[:, MAIN], DBL[:, 2:W + 2], 0.0, DBL[:, MAIN],
                        A.is_equal, A.logical_and)
                else:
                    nc.scalar.copy(DBL[:, 0:1], DBL[:, W:W + 1])
                    nc.vector.scalar_tensor_tensor(
                        AM8[:, MAIN], DBL[:, 0:W], 0.0, DBL[:, MAIN],
                        A.is_equal, A.logical_and)
                nc.scalar.copy(AM8[:, 0:1], AM8[:, W:W + 1])
                nc.scalar.copy(AM8[:, W + 1:W + 2], AM8[:, 1:2])
                s[f'A{which}8'] = AM8

            def blend(i, which):
                s = st[i]
                if which == 1:
                    src, dpool, dname = s['IN'], go1p, "go1"
                else:
                    src, dpool, dname = s['O1'], go2p, "go2"
                dst = dpool.tile([P, NCH, WH], I32, tag=dname,
                                 name=f"O{which}_{i}")
                s[f'O{which}'] = dst
                AM8 = s[f'A{which}8']
                if which == 1:
                    a_src, b_msk, b_src = slice(0, W), slice(2, W + 2), slice(2, W + 2)
                else:
                    a_src, b_msk, b_src = slice(2, W + 2), slice(0, W), slice(0, W)
                # plain copy on ACT as int16 (i16 -> f32 -> i16 is lossless, so
                # packed bf16 pairs and raw f32 bits survive), split in halves
                # so each predicated pair can start as soon as its half lands
                for c0, c1 in ((0, NCH // 2), (NCH // 2, NCH)):
                    nc.scalar.copy(dst[:, c0:c1, MAIN].bitcast(I16),
                                   src[:, c0:c1, MAIN].bitcast(I16))
                    nch = c1 - c0
                    am = AM8[:, MAIN].unsqueeze(1).broadcast_to((P, nch, W))
                    bm = AM8[:, b_msk].unsqueeze(1).broadcast_to((P, nch, W))
                    nc.vector.copy_predicated(dst[:, c0:c1, MAIN], am,
                                              src[:, c0:c1, a_src])
                    nc.vector.copy_predicated(dst[:, c0:c1, MAIN], bm,
                                              src[:, c0:c1, b_src])
                if which == 1:
                    nc.vector.tensor_copy(dst[:, :, 0:1], dst[:, :, W:W + 1])
                    nc.vector.tensor_copy(dst[:, :, W + 1:W + 2], dst[:, :, 1:2])
                else:
                    b, t = iters[i]
                    nc.sync.dma_start(out[b, t, :, 2:NST, :],
                                      dst[:, 2:NST, MAIN])
                    nc.sync.dma_start(out[b, t, :, 0:1, :], dst[:, 0:1, MAIN])

            def fixup(i):
                b, t = iters[i]
                s = st[i]
                O2 = s['O2']
                NF = mkp.tile([P, W], F32, tag="mk")
                FLI = am8p.tile([P, W], I8, tag="am8", name=f"FLI{i}")
                # nfm = 2*b1 - 2*b2 (masks exactly 0/1)
                nc.gpsimd.tensor_tensor(NF[:], s['A18'][:, 2:W + 2],
                                        s['A28'][:, 0:W], A.subtract)
                nc.gpsimd.tensor_scalar(NF[:], NF[:], 2.0, None, A.mult)
                nc.vector.tensor_copy(FLI[:], bv(O2, 11, 0, MAIN))
                nc.vector.copy_predicated(O2[:].bitcast(F32)[:, 1, MAIN],
                                          FLI[:], NF[:])
                nc.sync.dma_start(out[b, t, :, 1:2, :], O2[:, 1:2, MAIN])

            # ---- software-pipelined emission -------------------------------
            if order == 1:
                loads(0)
                loads(1)
                for i in range(n):
                    if i + 2 < n:
                        loads(i + 2)
                    mask_pass(i, 1)
                    blend(i, 1)
                    mask_pass(i, 2)
                    blend(i, 2)
                    fixup(i)
            elif order == 2:
                # two iterations' chains interleaved: mask chain of i+1 runs
                # while the blends of i occupy DVE/ACT
                loads(0)
                loads(1)
                mask_pass(0, 1)
                for i in range(n):
                    if i + 2 < n:
                        loads(i + 2)
                    blend(i, 1)
                    if i + 1 < n:
                        mask_pass(i + 1, 1)
                    mask_pass(i, 2)
                    blend(i, 2)
                    fixup(i)
            elif order == 4:
                # order 2, with the previous iteration's fixup deferred so it
                # fills DVE gaps during the next mask chain
                loads(0)
                loads(1)
                mask_pass(0, 1)
                for i in range(n):
                    if i + 2 < n:
                        loads(i + 2)
                    blend(i, 1)
                    if i + 1 < n:
                        mask_pass(i + 1, 1)
                    mask_pass(i, 2)
                    blend(i, 2)
                    if i > 0:
                        fixup(i - 1)
                fixup(n - 1)
            else:
                loads(0)
                loads(1)
                mask_pass(0, 1)
                blend(0, 1)
                for i in range(n):
                    if i + 2 < n:
                        loads(i + 2)
                    mask_pass(i, 2)
                    if i + 1 < n:
                        mask_pass(i + 1, 1)
                    blend(i, 2)
                    if i + 1 < n:
                        blend(i + 1, 1)
                    fixup(i)

    nc.compile()
    _nc_cache[key] = nc
    return nc


def _pack(lo, hi):
    return (_f32_to_bf16_bits(lo).astype(_u32)
            | (_f32_to_bf16_bits(hi).astype(_u32) << 16))


def prepare_inputs(world, rand_movement):
    """Pack the full-batch inputs into the device layout.

    Returns {"win": (B, N_HT, P, NCH, W) int32, "rand": (B, H, W) f32}.
    """
    ids = world[:, 0]
    E = np.zeros(ids.shape, np.float32)
    for v in FLUID_IDS:
        E += (ids == v)
    AIR = ((ids == 14.0) | (ids == 15.0)).astype(np.float32)
    ch = np.empty((B, NCH, H, W), _u32)
    ch[:, 0] = np.ascontiguousarray(world[:, 1]).view(_u32)
    ch[:, 1] = np.ascontiguousarray(world[:, 6]).view(_u32)
    for j, (lo, hi) in enumerate(PAIRS):
        ch[:, 2 + j] = _pack(world[:, lo], world[:, hi])
    ch[:, 11] = _pack(E, AIR)
    win = np.ascontiguousarray(
        ch.reshape(B, NCH, N_HT, P, W).transpose(0, 2, 3, 1, 4)).view(np.int32)
    return {"win": win, "rand": np.ascontiguousarray(rand_movement[:, 0])}


def unpack_output(stored):
    """(B, N_HT, P, NST, W) int32 device output -> (B, C, H, W) f32."""
    oc = stored.view(_u32).transpose(0, 3, 1, 2, 4).reshape(B, NST, H, W)
    full = np.empty((B, C, H, W), np.float32)
    full[:, 1] = np.ascontiguousarray(oc[:, 0]).view(np.float32)
    full[:, 6] = np.ascontiguousarray(oc[:, 1]).view(np.float32)
    for j, (lo, hi) in enumerate(PAIRS):
        c = np.ascontiguousarray(oc[:, 2 + j])
        full[:, lo] = _bf16_bits_to_f32((c & 0xFFFF).astype(_u16))
        full[:, hi] = _bf16_bits_to_f32((c >> 16).astype(_u16))
    return full


def kernel(world, rand_movement, rand_interact, rand_element):
    del rand_interact, rand_element
    nc = build_kernel()
    packed = prepare_inputs(np.asarray(world), np.asarray(rand_movement))
    in_maps = []
    for k in range(N_CORES):
        bs = slice(k * BPC, (k + 1) * BPC)
        in_maps.append({"win": packed["win"][bs], "rand": packed["rand"][bs]})
    res = run_bass_kernel_spmd(nc, in_maps, list(range(N_CORES)))
    stored = np.concatenate([res.results[k]["out"] for k in range(N_CORES)],
                            axis=0)
    return unpack_output(stored)
